# revision 18
# baseline (speedup 1.0000x reference)
"""2-layer GCN on 8 Trainium2 NeuronCores — v2.

Strategy (edge-cut node sharding):
- Core c owns dst nodes [c*12500, (c+1)*12500).
- Per layer, each core builds its shard of the message table
  tab[n] = (x @ W) * dinv[n] in fp16 (node-major [12500, 64], 128B rows),
  AllGathered so every core holds the full fp16 table [100000, 64] in HBM.
- Aggregation: edges (excl. self-loops, applied locally) are sorted by
  (512-dst window, 50000-node chunk, src parity, dst). dma_gather fetches
  node-PAIR rows (256B = 2 nodes x 64 fp16; pair id fits int16 per chunk).
  Per 128-slot tile (single parity), a one-hot [128, wd] is built on the DVE
  (is_equal vs iota, fp16 out) and a fp16 PE matmul accumulates
  psw[64, 512] += g[:, pair_sel].T @ onehot into the window's PSUM.
- dma_gather descriptor generation is the serial bottleneck on queue 0, but
  queues 1-3 retire async on their own Q7 core pairs: calls round-robin over
  queues (1,2,3,0) so 4 Q7 pairs generate descriptors concurrently.
- Window flush: * dinv[dst] + self-loop term + bias (+ ReLU for layer 1);
  layer-1 result hT [64, nodes] fp16 directly serves as lhsT for the layer-2
  table matmuls.
- SPMD: one program for all 8 cores; segment sizes are cross-core maxima;
  interior pads gather row 0 with dstcol=-1 (zero one-hot), trailing pads use
  idx=-1 which the gather ucode trims for free.
Host side does only sharding/layout/integer structure; float math on device.
"""
import numpy as np

N = 100000
E = 1600000
FIN = 128
HID = 64
FOUT = 64
NCORES = 8
NSH = N // NCORES            # 12500 nodes per core
NCHUNK = 2
CHS = N // NCHUNK            # 50000 nodes per chunk -> 25000 pairs (int16 ok)
WIN = 512                    # dst nodes per PSUM window
NW = (NSH + WIN - 1) // WIN  # 25 windows
NTILE_NODE = (NSH + 127) // 128  # 98 node tiles per core
GATHER_QUEUES = (1, 2, 3, 0)     # q1-3 retire async; q0 blocks Pool engine
SUBT = 16                        # tiles per gather call (2048 slots <= ring)


def _preprocess(edge_index):
    src = np.asarray(edge_index[0], dtype=np.int64)
    dst = np.asarray(edge_index[1], dtype=np.int64)
    deg = (np.bincount(dst, minlength=N) + 1).astype(np.float32)

    percore = []
    for c in range(NCORES):
        lo, hi = c * NSH, (c + 1) * NSH
        sel = (dst >= lo) & (dst < hi)
        s, d = src[sel], dst[sel] - lo
        w = d // WIN
        b = s // CHS
        p = s & 1
        order = np.lexsort((d, p, b, w))
        s, d, w, b, p = s[order], d[order], w[order], b[order], p[order]
        cnt = np.zeros((NW, NCHUNK, 2), np.int64)
        np.add.at(cnt, (w, b, p), 1)
        percore.append((s, d, cnt))

    cnts = np.stack([pc[2] for pc in percore])          # [8, NW, NCHUNK, 2]
    seg = ((cnts.max(axis=0) + 127) // 128) * 128       # [NW, NCHUNK, 2]
    S_wb = seg.sum(axis=2)                              # slots per (w, chunk)
    starts = np.concatenate([[0], np.cumsum(S_wb.reshape(-1))]).astype(np.int64)
    total_slots = int(starts[-1])

    gidx = np.zeros((NCORES, total_slots), np.int64)       # pair id (pad 0)
    dcol = np.full((NCORES, total_slots), -1.0, np.float32)
    rawcol = np.full((NCORES, total_slots), -1, np.int64)
    for c in range(NCORES):
        s, d, cnt = percore[c]
        pos = 0
        for w in range(NW):
            for b in range(NCHUNK):
                base = starts[w * NCHUNK + b]
                off = 0
                for p in range(2):
                    n = cnt[w, b, p]
                    sl = slice(base + off, base + off + n)
                    gidx[c, sl] = (s[pos:pos + n] - b * CHS) >> 1
                    rawcol[c, sl] = d[pos:pos + n] - w * WIN
                    pos += n
                    off += seg[w, b, p]
                # pads keep gidx 0 (valid row): the gather ucode requires the
                # runtime num_idxs reg == count of non-negative indices, which
                # must be the compile-time S in an SPMD program
        assert pos == len(s)
    dcol[rawcol >= 0] = rawcol[rawcol >= 0].astype(np.float32)

    # per (w,b): per-tile parity + list of (o, wd) one-hot sub-windows
    sched = []            # per call: (ntiles, [parity_j], [offs_j])
    for w in range(NW):
        c1 = min(NSH, w * WIN + WIN)
        ncol = c1 - w * WIN
        for b in range(NCHUNK):
            gi = w * NCHUNK + b
            base = starts[gi]
            nt = int(S_wb[w, b]) // 128
            nt0 = int(seg[w, b, 0]) // 128
            pars, offs = [], []
            for j in range(nt):
                pars.append(0 if j < nt0 else 1)
                colseg = rawcol[:, base + j * 128: base + (j + 1) * 128]
                real = colseg[colseg >= 0]
                if real.size == 0:
                    offs.append([])
                    continue
                lo_c, hi_c = int(real.min()), int(real.max())
                sub = []
                o = lo_c
                while o <= hi_c:
                    wd = min(128, hi_c + 1 - o, ncol - o)
                    sub.append((o, wd))
                    o += 128
                offs.append(sub)
            sched.append((nt, pars, offs))

    def wrap16(flat):
        n = flat.shape[0]
        wtile = flat.reshape(n // 16, 16).T.astype(np.int16)
        return np.tile(wtile, (8, 1))

    gidx_w = np.stack([wrap16(gidx[c]) for c in range(NCORES)])
    dcol_b = dcol.reshape(NCORES, total_slots // 128, 128).transpose(
        0, 2, 1).copy()
    return deg, gidx_w, dcol_b, S_wb, starts, sched, total_slots


def _build_program(S_wb, starts, sched, total_slots):
    from concourse import bass, bacc, mybir, tile

    f32 = mybir.dt.float32
    f16 = mybir.dt.float16
    i16 = mybir.dt.int16
    nc = bacc.Bacc(None, target_bir_lowering=False, num_swdge_queues=4)

    xT = nc.dram_tensor("xT", [FIN, NSH], f16, kind="ExternalInput")
    W1 = nc.dram_tensor("W1", [FIN, HID], f16, kind="ExternalInput")
    W2 = nc.dram_tensor("W2", [HID, FOUT], f16, kind="ExternalInput")
    b1 = nc.dram_tensor("b1", [HID, 1], f32, kind="ExternalInput")
    b2 = nc.dram_tensor("b2", [FOUT, 1], f32, kind="ExternalInput")
    degT = nc.dram_tensor("deg", [128, NTILE_NODE], f32, kind="ExternalInput")
    gidxT = nc.dram_tensor("gidx", [128, total_slots // 16], i16,
                           kind="ExternalInput")
    dcolT = nc.dram_tensor("dcol", [128, total_slots // 128], f32,
                           kind="ExternalInput")
    outT = nc.dram_tensor("out", [FOUT, NSH], f32, kind="ExternalOutput")

    tab_my = [nc.dram_tensor(f"tab_my{l}", [NSH, HID], f16) for l in (1, 2)]
    tab_full = [nc.dram_tensor(f"tab_full{l}", [N, HID], f16,
                               addr_space="Shared") for l in (1, 2)]
    dinv_dram = nc.dram_tensor("dinv_dram", [NTILE_NODE * 128], f32)

    iota_np = np.tile(np.arange(WIN, dtype=np.float32), (128, 1))
    iota_dram = nc.inline_tensor(iota_np, name="iota512")

    rg = [list(range(NCORES))]

    with tile.TileContext(nc) as tc:
        with (
            tc.tile_pool(name="const", bufs=1) as cpool,
            tc.tile_pool(name="hT", bufs=1) as hpool,
            tc.tile_pool(name="dinvb", bufs=1) as dbpool,
            tc.tile_pool(name="mm", bufs=3) as mmpool,
            tc.tile_pool(name="psA", bufs=2, space="PSUM") as psA,
            tc.tile_pool(name="psB", bufs=2, space="PSUM") as psB,
        ):
            w1t = cpool.tile([FIN, HID], f16)
            nc.sync.dma_start(out=w1t[:], in_=W1[:, :])
            w2t = cpool.tile([HID, FOUT], f16)
            nc.sync.dma_start(out=w2t[:], in_=W2[:, :])
            b1t = cpool.tile([HID, 1], f32)
            nc.sync.dma_start(out=b1t[:], in_=b1[:, :])
            b2t = cpool.tile([FOUT, 1], f32)
            nc.sync.dma_start(out=b2t[:], in_=b2[:, :])
            iot = cpool.tile([128, WIN], f32)
            nc.sync.dma_start(out=iot[:], in_=iota_dram[:, :])
            degt = cpool.tile([128, NTILE_NODE], f32)
            nc.sync.dma_start(out=degt[:], in_=degT[:, :])
            dsq = cpool.tile([128, NTILE_NODE], f32)
            nc.scalar.activation(dsq[:], degt[:],
                                 mybir.ActivationFunctionType.Sqrt)
            dinv = cpool.tile([128, NTILE_NODE], f32)
            nc.vector.reciprocal(dinv[:], dsq[:])
            nc.sync.dma_start(
                out=dinv_dram.ap().rearrange("(t p) -> p t", p=128), in_=dinv[:])
            dinvb = dbpool.tile([HID, NSH], f32)
            nc.sync.dma_start(out=dinvb[:1, :], in_=dinv_dram.ap()[None, :NSH])
            k = 1
            while k < HID:
                kk = min(k, HID - k)
                nc.sync.dma_start(out=dinvb[k:k + kk, :], in_=dinvb[:kk, :])
                k += kk

            zt = cpool.tile([128, WIN], f16)
            nc.vector.memset(zt[:], 0.0)
            hT = hpool.tile([HID, NTILE_NODE * 128], f16)
            selfT = dbpool.tile([HID, NSH], f16)

            # all gather indices + dst cols resident in SBUF (used by both
            # layers; avoids reuse races with async-queue descriptor gen)
            itall = cpool.tile([128, total_slots // 16], i16)
            nc.sync.dma_start(out=itall[:], in_=gidxT[:, :])
            dtall = cpool.tile([128, total_slots // 128], f32)
            nc.sync.dma_start(out=dtall[:], in_=dcolT[:, :])

            # ---- layer-1 table: tab_my1[n] = (x @ W1)[n] * dinv[n] ----
            with tc.tile_pool(name="xT", bufs=3) as xpool:
                for t in range(NTILE_NODE):
                    n0 = t * 128
                    n1 = min(NSH, n0 + 128)
                    nn = n1 - n0
                    xt = xpool.tile([FIN, 128], f16)
                    nc.sync.dma_start(out=xt[:, :nn], in_=xT[:, n0:n1])
                    ps = psA.tile([128, HID], f32, space="PSUM")
                    nc.tensor.matmul(ps[:nn, :], lhsT=xt[:, :nn], rhs=w1t[:],
                                     start=True, stop=True)
                    sb = mmpool.tile([128, HID], f16)
                    nc.vector.tensor_scalar_mul(sb[:nn, :], ps[:nn, :],
                                                dinv[:nn, t:t + 1])
                    nc.sync.dma_start(out=tab_my[0][n0:n1, :], in_=sb[:nn, :])
                    psT = psB.tile([HID, 128], f32, space="PSUM")
                    nc.tensor.matmul(psT[:, :nn], lhsT=w1t[:], rhs=xt[:, :nn],
                                     start=True, stop=True)
                    nc.vector.tensor_mul(selfT[:, n0:n1], psT[:, :nn],
                                         dinvb[:, n0:n1])
                    nc.vector.tensor_mul(selfT[:, n0:n1], selfT[:, n0:n1],
                                         dinvb[:, n0:n1])

            nc.gpsimd.collective_compute(
                "AllGather", mybir.AluOpType.bypass, replica_groups=rg,
                ins=[tab_my[0].ap().opt()], outs=[tab_full[0].ap().opt()])

            # ---- aggregation layers ----
            qi = 0
            for layer in (0, 1):
                tabf = tab_full[layer]
                with (
                    tc.tile_pool(name=f"gb{layer}", bufs=3) as gpool,
                    tc.tile_pool(name=f"oh{layer}", bufs=6) as ohpool,
                    tc.tile_pool(name=f"fl{layer}", bufs=2) as flpool,
                    tc.tile_pool(name=f"psW{layer}", bufs=2, space="PSUM") as psW,
                ):
                    for w in range(NW):
                        c0 = w * WIN
                        c1 = min(NSH, c0 + WIN)
                        ncol = c1 - c0
                        psw = psW.tile([HID, WIN], f32, space="PSUM")
                        n_mm = sum(len(sched[w * NCHUNK + bb][2][j])
                                   for bb in range(NCHUNK)
                                   if S_wb[w, bb] > 0
                                   for j in range(sched[w * NCHUNK + bb][0]))
                        nc.tensor.matmul(psw[:], lhsT=w1t[:], rhs=zt[:],
                                         start=True, stop=(n_mm == 0))
                        mm_i = 0
                        for b in range(NCHUNK):
                            gi = w * NCHUNK + b
                            S = int(S_wb[w, b])
                            if S == 0:
                                continue
                            base = int(starts[gi])
                            nt, pars, offs = sched[gi]
                            chunk = tabf[b * CHS:(b + 1) * CHS, :].rearrange(
                                "(n two) f -> n (two f)", two=2)
                            for j0 in range(0, nt, SUBT):
                                ns = min(SUBT, nt - j0)
                                Ss = ns * 128
                                b0 = base + j0 * 128
                                it = itall[:, b0 // 16: b0 // 16 + Ss // 16]
                                g = gpool.tile([128, SUBT * 128], f16)
                                nc.gpsimd.dma_gather(
                                    g[:, :ns * 128].rearrange(
                                        "p (n f) -> p n f", n=ns),
                                    chunk, it, Ss, Ss, 2 * HID,
                                    single_packet=False,
                                    queue_num=GATHER_QUEUES[qi % 4])
                                qi += 1
                                for j in range(j0, j0 + ns):
                                    p = pars[j]
                                    lhs = g[:, (j - j0) * 128 + p * HID:
                                            (j - j0) * 128 + p * HID + HID]
                                    jc = base // 128 + j
                                    for (o, wd) in offs[j]:
                                        oh = ohpool.tile([128, 128], f16)
                                        nc.vector.tensor_tensor(
                                            oh[:, :wd],
                                            dtall[:, jc:jc + 1].to_broadcast(
                                                [128, wd]),
                                            iot[:, o:o + wd],
                                            mybir.AluOpType.is_equal)
                                        mm_i += 1
                                        nc.tensor.matmul(
                                            psw[:, o:o + wd], lhsT=lhs,
                                            rhs=oh[:, :wd], start=False,
                                            stop=(mm_i == n_mm))
                        fl = flpool.tile([HID, WIN], f32)
                        nc.vector.tensor_mul(fl[:, :ncol], psw[:, :ncol],
                                             dinvb[:, c0:c1])
                        nc.vector.tensor_add(fl[:, :ncol], fl[:, :ncol],
                                             selfT[:, c0:c1])
                        if layer == 0:
                            nc.scalar.activation(
                                hT[:, c0:c1], fl[:, :ncol],
                                mybir.ActivationFunctionType.Relu,
                                bias=b1t[:])
                        else:
                            nc.vector.tensor_tensor(
                                fl[:, :ncol], fl[:, :ncol],
                                b2t[:].to_broadcast([FOUT, ncol]),
                                mybir.AluOpType.add)
                            nc.sync.dma_start(out=outT[:, c0:c1],
                                              in_=fl[:, :ncol])

                if layer == 0:
                    # layer-2 table + self term
                    for t in range(NTILE_NODE):
                        n0 = t * 128
                        n1 = min(NSH, n0 + 128)
                        nn = n1 - n0
                        ps = psA.tile([128, FOUT], f32, space="PSUM")
                        nc.tensor.matmul(ps[:nn, :], lhsT=hT[:, n0:n1][:, :nn],
                                         rhs=w2t[:], start=True, stop=True)
                        sb = mmpool.tile([128, FOUT], f16)
                        nc.vector.tensor_scalar_mul(sb[:nn, :], ps[:nn, :],
                                                    dinv[:nn, t:t + 1])
                        nc.sync.dma_start(out=tab_my[1][n0:n1, :],
                                          in_=sb[:nn, :])
                        psT = psB.tile([FOUT, 128], f32, space="PSUM")
                        nc.tensor.matmul(psT[:, :nn], lhsT=w2t[:],
                                         rhs=hT[:, n0:n1][:, :nn],
                                         start=True, stop=True)
                        nc.vector.tensor_mul(selfT[:, n0:n1], psT[:, :nn],
                                             dinvb[:, n0:n1])
                        nc.vector.tensor_mul(selfT[:, n0:n1],
                                             selfT[:, n0:n1],
                                             dinvb[:, n0:n1])
                    nc.gpsimd.collective_compute(
                        "AllGather", mybir.AluOpType.bypass, replica_groups=rg,
                        ins=[tab_my[1].ap().opt()],
                        outs=[tab_full[1].ap().opt()])
    nc.compile()
    return nc


TRACE = False
_LAST_TIMING = None


def kernel(x, edge_index, W1, b1, W2, b2):
    from concourse.bass_utils import run_bass_kernel_spmd

    x = np.asarray(x, np.float32)
    W1 = np.asarray(W1, np.float32)
    W2 = np.asarray(W2, np.float32)
    b1 = np.asarray(b1, np.float32)
    b2 = np.asarray(b2, np.float32)

    deg, gidx_w, dcol_b, S_wb, starts, sched, total_slots = \
        _preprocess(edge_index)

    nc = _build_program(S_wb, starts, sched, total_slots)

    in_maps = []
    for c in range(NCORES):
        lo, hi = c * NSH, (c + 1) * NSH
        degp = np.ones(NTILE_NODE * 128, np.float32)
        degp[:NSH] = deg[lo:hi]
        in_maps.append({
            "xT": np.ascontiguousarray(x[lo:hi].T).astype(np.float16),
            "W1": W1.astype(np.float16), "W2": W2.astype(np.float16),
            "b1": b1.reshape(HID, 1), "b2": b2.reshape(FOUT, 1),
            "deg": np.ascontiguousarray(degp.reshape(NTILE_NODE, 128).T),
            "gidx": gidx_w[c],
            "dcol": dcol_b[c],
        })

    kwargs = {"trace": True} if TRACE else {}
    res = run_bass_kernel_spmd(nc, in_maps, core_ids=list(range(NCORES)),
                               **kwargs)
    globals()["_LAST_TIMING"] = getattr(res, "exec_time_ns", None)

    z = np.empty((N, FOUT), np.float32)
    for c in range(NCORES):
        lo, hi = c * NSH, (c + 1) * NSH
        z[lo:hi] = np.asarray(res.results[c]["out"]).reshape(FOUT, NSH).T
    return z
